# revision 40
# baseline (speedup 1.0000x reference)
"""Single-head attention kernel for Trainium2 (Bass/Tile), 8 NeuronCores.

Problem: B=4, S=4096, D=1024, H=128 fp32.
    q,k,v = x @ W{q,k,v};  out = softmax(q k^T / sqrt(H)) @ v

Sharding: 8 cores = (batch b, KEY-half kh).  Each core computes PARTIAL
attention for all 4096 queries over its 2048 keys; the host combines the
two partial results per batch: out = (outT_0 + outT_1) / (l_0 + l_1)
(unnormalized value-sums and softmax denominators add across key shards).
The host permutes each core's x rows so its key rows come first and
transposes/casts to xT fp16, laid out slice-contiguously ([p, g, c, s])
so each 512-row slice load is one 2D DMA descriptor per partition.
Query order follows the same permutation; the host maps it back.

fp16 (e5m10) everywhere on the matmul operands: 2-byte operands stream
through the PE at 1 column/cycle @ 2.4 GHz and the 10-bit mantissa keeps
end-to-end error ~5e-4 (all tensors here are O(1)).  Accumulation is
fp32 in PSUM.

Per-core dataflow (PSUM: 1 bank proj + 4 banks scores + 2 banks outT +
1 bank row-sums = 8):
  1. PE clock warm-up (HAM) on a memset tile from t~0 -- no DMA
     dependency.  xT k-slices DMA'd in slice-major, split across DMA
     queues; wq + slice 0 first.
  2. Projections per 512-row block through rotating PSUM banks; the
     first blocks borrow idle attention PSUM slots.  qT for all 8 query
     blocks -- blocks 4-7 are deferred and interleaved one dc-matmul at
     a time through chunks 0-2's kb loops, so each matmul fills a PE
     wait at the exp-bound score-bank ping-pong; kT/vT for the 4
     own-key blocks; vT PE-transposed.
  3. Scores TRANSPOSED, 1024-query chunks: sT[k,q] = kT(kb)^T @ qT.
     ScalarE exp reads sT from PSUM, writes attnT (fp16) to SBUF, and
     stays a pure gapless exp chain -- it is the per-chunk bottleneck.
     Chunk 0's first four score tiles (needing only q0,q1,kT block 0)
     are emitted BEFORE the remaining projections, starting the exp
     chain ~6us earlier under the projection loop.  No max
     subtraction: scores ~N(0,1), fp32 exp is safe.
  4. outT[h,q] += v[kb]^T @ attnT on the PE.  Row-sums l[q]: attnT
     tiles pair/quad/oct tree-summed on the (otherwise idle) DVE in
     fp16, then just 2 ones-matmuls per oct tile -- 4x fewer PE
     columns than per-pair ones-matmuls.  The last chunk sums its
     second half at pair granularity instead: one DVE add after the
     final exp instead of three, shortening the kernel's tail.
  5. Partial outT and l DMA'd out (l first -- longest chain; all
     evacuation on the DVE); host combines shards + normalizes.

The body is PE-bound (~100us busy, span ~105.8us with only head-DMA
waits left; ScalarE exp ~71us ends co-terminal; ~13us is fixed NEFF
epilogue + evacuation tail).
Tried and reverted: pairwise-AllGather qT dedup (collective latency
~35us + DMA contention exceed the saved matmuls), DMA-xbar v
transposes (SP-sequencer issue serialization stalls the AV chain),
fp8 matmuls (2e-2 error gate; fp8 scores alone cost ~2.5e-2).
"""

import math

import numpy as np

import concourse.bacc as bacc
import concourse.mybir as mybir
import concourse.tile as tile
from concourse.bass_utils import run_bass_kernel_spmd

B, S, D, H = 4, 4096, 1024, 128
NCORES = 8
SK = S // 2  # keys per core (2048)
RB = 512  # rows per projection block
NRB = S // RB  # 8 query blocks
NKRB = SK // RB  # 4 key blocks
QC = 1024  # queries per attention chunk
NQC = S // QC  # 4 chunks
NKB = SK // 128  # 16 key blocks of 128
NDC = D // 128  # 8 contraction chunks

F32 = mybir.dt.float32
F16 = mybir.dt.float16

_CACHE = {}


def build_nc():
    nc = bacc.Bacc("TRN2", target_bir_lowering=False, debug=False)

    # x^T slice-contiguous: [p, g, c, s] = xT[c*128+p, g*512+s]
    xt_d = nc.dram_tensor("xt", [128, NRB, NDC, RB], F16, kind="ExternalInput")
    # weights host-preswizzled to [128, NDC*H]: row p, chunk c = W[c*128+p, :]
    wq_d = nc.dram_tensor("wq", [128, NDC * H], F16, kind="ExternalInput")
    wk_d = nc.dram_tensor("wk", [128, NDC * H], F16, kind="ExternalInput")
    wv_d = nc.dram_tensor("wv", [128, NDC * H], F16, kind="ExternalInput")
    ident_d = nc.dram_tensor("ident", [128, 128], F16, kind="ExternalInput")
    ones_d = nc.dram_tensor("ones", [128, 1], F16, kind="ExternalInput")
    # partial (key-shard) unnormalized out^T [h, q] and denominators l [1, q]
    outT_d = nc.dram_tensor("outT", [H, S], F32, kind="ExternalOutput")
    l_d = nc.dram_tensor("l", [1, S], F32, kind="ExternalOutput")
    # (A pairwise AllGather exchanging half the qT projection with the
    # sibling core was tried and reverted: the collective's ~35us latency
    # plus its DMA contention with PE streams cost more than the 3.5us of
    # saved projection matmuls.)

    scale = 1.0 / math.sqrt(H)

    with tile.TileContext(nc) as tc:
        with (
            tc.tile_pool(name="const", bufs=1) as constp,
            tc.tile_pool(name="persist", bufs=1) as persist,
            tc.tile_pool(name="attn", bufs=6) as attn_pool,
            tc.tile_pool(name="fin", bufs=2) as fin_pool,
            tc.tile_pool(name="ps_p", bufs=1, space="PSUM") as ps_p,
            tc.tile_pool(name="ps_s", bufs=2, space="PSUM") as ps_s,
            tc.tile_pool(name="ps_o", bufs=1, space="PSUM") as ps_o,
        ):
            # ---- DMA, ordered for the critical path ----
            w_sb = {}
            for name in ("wq", "wk", "wv"):
                w_sb[name] = constp.tile([128, NDC, H], F16, name=f"{name}_sb")

            def load_w(name):
                nc.sync.dma_start(
                    w_sb[name][:],
                    {"wq": wq_d, "wk": wk_d, "wv": wv_d}[name]
                    .ap()
                    .rearrange("p (c h) -> p c h", c=NDC),
                )

            xt_sb = persist.tile([128, NDC, S], F16, name="xt_sb")

            def load_slice(g, nsplit=2):
                # split dma_starts land on different queues -> slice arrives
                # in ~1/nsplit the wall time (key slices gate the projection
                # front, so they get a finer split)
                w = NDC // nsplit
                for ch in range(nsplit):
                    nc.sync.dma_start(
                        xt_sb[:, ch * w : (ch + 1) * w, g * RB : (g + 1) * RB],
                        xt_d.ap()[:, g, ch * w : (ch + 1) * w],
                    )

            ident = constp.tile([128, 128], F16, name="ident_sb")
            ones = constp.tile([128, 1], F16, name="ones_sb")

            load_w("wq")
            load_slice(0, nsplit=4)
            load_w("wk")
            load_w("wv")
            nc.sync.dma_start(ident[:], ident_d.ap())
            nc.sync.dma_start(ones[:], ones_d.ap())
            for g in range(1, NRB):
                load_slice(g, nsplit=4 if g < NKRB else 2)

            # ---- persistent activations ----
            qt_sb = persist.tile([128, S], F16, name="qt_sb")  # [h, q] all q
            kt_sb = persist.tile([128, SK], F16, name="kt_sb")  # [h, k] own
            v_sb = persist.tile([128, NKB, H], F16, name="v_sb")  # own keys
            vt_sb = persist.tile([128, SK], F16, name="vt_sb")  # staging

            # preload the exp table during the input DMA (reads a memset
            # tile, not a DMA'd one, so it issues immediately)
            warm_w = constp.tile([128, 128], F16, name="warm_w")
            nc.vector.memset(warm_w[:], 0.0)
            warm = constp.tile([1, 1], F32, name="warm_sb")
            nc.scalar.activation(
                warm[:], warm_w[0:1, 0:1], mybir.ActivationFunctionType.Exp
            )
            # HAM warm-up on the memset tile (no DMA dependency).  Short on
            # purpose: the 4096-cycle HAM activity window only needs the PE
            # busy from t~0 -- the first real projections continue the
            # activity and the clock reaches 2.4 GHz ~3.4us in either way.
            # A long warmup chain would block the in-order PE queue past
            # slice 0's arrival (~2.5us) and delay the first projection.
            warm_ps = ps_p.tile([128, 128], F32, tag="proj")
            for i in range(30):
                nc.tensor.matmul(
                    warm_ps[:],
                    warm_w[:],
                    warm_w[:],
                    start=(i == 0),
                    stop=(i == 29),
                )

            def project(wname, dst_sb, rb, pool, tag, width):
                """One 512-row projection block through one PSUM bank."""
                ps = pool.tile([128, width], F32, tag=tag)
                for dc in range(NDC):
                    nc.tensor.matmul(
                        ps[:, 0:RB],
                        w_sb[wname][:, dc, :],
                        xt_sb[:, dc, rb * RB : (rb + 1) * RB],
                        start=(dc == 0),
                        stop=(dc == NDC - 1),
                    )
                nc.vector.tensor_copy(dst_sb[:, rb * RB : (rb + 1) * RB], ps[:, 0:RB])

            def v_transpose(g):
                v_ps = ps_p.tile([128, RB], F16, tag="proj")
                for s in range(4):
                    nc.tensor.transpose(
                        v_ps[:, s * 128 : (s + 1) * 128],
                        vt_sb[:, g * RB + s * 128 : g * RB + (s + 1) * 128],
                        ident[:],
                    )
                nc.vector.tensor_copy(
                    v_sb[:, g * 4 : (g + 1) * 4, :].rearrange("p a b -> p (a b)"),
                    v_ps[:, 0 : 4 * H],
                )

            def score_tile(qcidx, kb, at_tiles):
                """Score+exp for one (chunk, key-block): sT = kT(kb)^T @ qT,
                then ScalarE exp PSUM->SBUF fp16."""
                st_ps = ps_s.tile([128, QC], F32, tag="st")
                for h in range(QC // 512):
                    nc.tensor.matmul(
                        st_ps[:, h * 512 : (h + 1) * 512],
                        kt_sb[:, kb * 128 : (kb + 1) * 128],
                        qt_sb[
                            :, qcidx * QC + h * 512 : qcidx * QC + (h + 1) * 512
                        ],
                        start=True,
                        stop=True,
                    )
                at_sb = attn_pool.tile([128, QC], F16, tag="at")
                nc.scalar.activation(
                    at_sb[:],
                    st_ps[:],
                    mybir.ActivationFunctionType.Exp,
                    scale=scale,
                )
                at_tiles[kb] = at_sb

            # Front: blocks attention chunk 0 needs, accumulated in parallel
            # on idle attention PSUM slots.
            project("wq", qt_sb, 0, ps_s, "st", QC)
            project("wk", kt_sb, 0, ps_s, "st", QC)
            project("wv", vt_sb, 0, ps_o, "outT", QC)
            project("wq", qt_sb, 1, ps_o, "l", 512)
            # Chunk 0's first four score tiles need only qT blocks 0,1 and
            # kT block 0 -- emit them BEFORE the remaining projections so
            # the ScalarE exp chain (the chunk-phase bottleneck) starts
            # ~6us earlier and overlaps the projection loop.
            at_tiles_c0 = {}
            for kb in range(4):
                score_tile(0, kb, at_tiles_c0)
            # tr0 has no slice-1 dependency -- emitted after the pre-scores
            # so it doesn't sit between q1 and the exp-chain start
            v_transpose(0)
            # Rest through the 1-bank proj slot; the scheduler overlaps with
            # the attention chain.
            for g in range(1, NKRB):
                project("wk", kt_sb, g, ps_p, "proj", RB)
                project("wv", vt_sb, g, ps_p, "proj", RB)
                v_transpose(g)
                if g < NKRB - 1:
                    project("wq", qt_sb, g + 1, ps_p, "proj", RB)

            # ---- attention (software-pipelined by one kb) ----
            # Row-sums l: attnT tiles are tree-summed (pairs -> quads ->
            # octs) on the idle DVE in fp16 (sums stay < 1e4, well within
            # fp16 range); the PE then only streams 2 ones-matmuls per oct
            # tile -- 4x fewer PE columns than summing per-pair.
            for qcidx in range(NQC):
                outT_ps = ps_o.tile([128, QC], F32, tag="outT")
                # both 512-halves of l packed into ONE psum bank (partition 0
                # and partition 32 via tile_position col 32)
                l_ps = ps_o.tile([64, 512], F32, tag="l")
                at_tiles = at_tiles_c0 if qcidx == 0 else {}

                # deferred qT blocks spread at single-matmul granularity
                # through this chunk's kb loop: the chunk phase is ACT
                # (exp)-bound, so each dc-matmul slots into a PE wait that
                # would otherwise be lost at the score-bank ping-pong
                def_blocks = {0: [4, 5], 1: [6], 2: [7]}.get(qcidx, [])
                def_items = iter(
                    [(rb, dc) for rb in def_blocks for dc in range(NDC)]
                )
                def_ps = {}

                def emit_deferred(n):
                    for _ in range(n):
                        item = next(def_items, None)
                        if item is None:
                            return
                        rb, dc = item
                        if dc == 0:
                            def_ps[rb] = ps_p.tile(
                                [128, RB], F32, tag="proj", name=f"defq{rb}"
                            )
                        nc.tensor.matmul(
                            def_ps[rb][:, :],
                            w_sb["wq"][:, dc, :],
                            xt_sb[:, dc, rb * RB : (rb + 1) * RB],
                            start=(dc == 0),
                            stop=(dc == NDC - 1),
                        )
                        if dc == NDC - 1:
                            nc.vector.tensor_copy(
                                qt_sb[:, rb * RB : (rb + 1) * RB],
                                def_ps.pop(rb)[:, :],
                            )

                def score(kb):
                    if kb in at_tiles:  # chunk 0's kb 0-3: emitted early
                        return
                    score_tile(qcidx, kb, at_tiles)

                def accum_av(kb):
                    at_sb = at_tiles[kb]
                    for h in range(QC // 512):
                        nc.tensor.matmul(
                            outT_ps[:, h * 512 : (h + 1) * 512],
                            v_sb[:, kb, :],
                            at_sb[:, h * 512 : (h + 1) * 512],
                            start=(kb == 0),
                            stop=(kb == NKB - 1),
                        )

                pair_tiles = {}
                quad_tiles = {}
                oct_tiles = {}

                def pair_add(p):
                    a = at_tiles[2 * p]
                    b = at_tiles[2 * p + 1]
                    pair = attn_pool.tile([128, QC], F16, tag="pair", bufs=3)
                    nc.vector.tensor_add(pair[:], a[:], b[:])
                    pair_tiles[p] = pair

                def quad_add(q):
                    a = pair_tiles.pop(2 * q)
                    b = pair_tiles.pop(2 * q + 1)
                    quad = attn_pool.tile([128, QC], F16, tag="quad", bufs=2)
                    nc.vector.tensor_add(quad[:], a[:], b[:])
                    quad_tiles[q] = quad

                def oct_add(o):
                    a = quad_tiles.pop(2 * o)
                    b = quad_tiles.pop(2 * o + 1)
                    oct = attn_pool.tile([128, QC], F16, tag="oct", bufs=2)
                    nc.vector.tensor_add(oct[:], a[:], b[:])
                    oct_tiles[o] = oct

                last = qcidx == NQC - 1

                def ones_mm(t, is_start, is_stop):
                    for h in range(QC // 512):
                        nc.tensor.matmul(
                            l_ps[h * 32 : h * 32 + 1, :],
                            ones[:],
                            t[:, h * 512 : (h + 1) * 512],
                            start=is_start,
                            stop=is_stop,
                            tile_position=(0, h * 32),
                        )

                # In the last chunk the second half of l is summed at PAIR
                # granularity: the post-exp dependency chain shrinks from
                # pair+quad+oct adds (~2.1us DVE latency) to one pair add,
                # pulling the final l DMA (the kernel's last output) earlier.
                score(0)
                for kb in range(1, NKB):
                    score(kb)
                    if kb % 2 == 1:
                        pair_add((kb - 1) // 2)
                    if not (last and kb > 7):
                        if kb % 4 == 3:
                            quad_add((kb - 3) // 4)
                        if kb % 8 == 7:
                            oct_add((kb - 7) // 8)
                    accum_av(kb - 1)
                    emit_deferred(1 if qcidx == 0 else (kb % 2))
                    if kb == 10:
                        ones_mm(oct_tiles.pop(0), True, False)
                    if last and kb in (12, 14):
                        ones_mm(pair_tiles.pop((kb - 12) // 2 + 4), False, False)
                    if last and kb == 15:
                        ones_mm(pair_tiles.pop(6), False, False)
                accum_av(NKB - 1)
                emit_deferred(NDC * len(def_blocks))  # drain any remainder
                if last:
                    ones_mm(pair_tiles.pop(7), False, True)
                else:
                    ones_mm(oct_tiles.pop(1), False, True)

                # evacuate row-sums first (the l path is the longest
                # post-exp chain), then partial outT; all evacuation on the
                # DVE -- the ScalarE stays a pure exp chain
                l_sb = fin_pool.tile([1, QC], F32, tag="l_sb")
                # (ScalarE l copies for the last chunk were tried: the [1,512]
                # copies hit the ScalarE SBUF-op errata floor and extended the
                # ACT chain past the DVE path -- DVE copies are faster.)
                l_copy = nc.vector.tensor_copy
                l_copy(l_sb[:, 0:512], l_ps[0:1, :])
                nc.sync.dma_start(
                    l_d.ap()[:, qcidx * QC : qcidx * QC + 512], l_sb[:, 0:512]
                )
                l_copy(l_sb[:, 512:1024], l_ps[32:33, :])
                nc.sync.dma_start(
                    l_d.ap()[:, qcidx * QC + 512 : (qcidx + 1) * QC],
                    l_sb[:, 512:1024],
                )
                outT_sb = fin_pool.tile([128, QC], F32, tag="outT_sb")
                if last:
                    # final chunk: evacuate in halves so the DMA of the
                    # first half overlaps the copy of the second
                    for hh in range(2):
                        nc.vector.tensor_copy(
                            outT_sb[:, hh * 512 : (hh + 1) * 512],
                            outT_ps[:, hh * 512 : (hh + 1) * 512],
                        )
                        nc.sync.dma_start(
                            outT_d.ap()[
                                :, qcidx * QC + hh * 512 : qcidx * QC + (hh + 1) * 512
                            ],
                            outT_sb[:, hh * 512 : (hh + 1) * 512],
                        )
                else:
                    nc.vector.tensor_copy(outT_sb[:], outT_ps[:])
                    nc.sync.dma_start(
                        outT_d.ap()[:, qcidx * QC : (qcidx + 1) * QC], outT_sb[:]
                    )



    nc.compile()
    return nc


def _get_nc():
    if "nc" not in _CACHE:
        _CACHE["nc"] = build_nc()
    return _CACHE["nc"]


def _swizzle_w(W):
    # [D, H] -> [128, NDC*H]: row p, chunk c holds W[c*128+p, :]
    W = np.asarray(W, dtype=np.float16)
    return np.ascontiguousarray(
        W.reshape(NDC, 128, H).transpose(1, 0, 2).reshape(128, NDC * H)
    )


def make_in_maps(inputs, Wq, Wk, Wv):
    inputs = np.asarray(inputs, dtype=np.float32)
    Wq = _swizzle_w(Wq)
    Wk = _swizzle_w(Wk)
    Wv = _swizzle_w(Wv)
    ident = np.eye(128, dtype=np.float16)
    ones = np.ones((128, 1), dtype=np.float16)

    in_maps = []
    for c in range(NCORES):
        b, kh = divmod(c, 2)
        xb = inputs[b]
        # own key-half rows first; queries follow the same permutation
        xk = np.concatenate(
            [xb[kh * SK : (kh + 1) * SK], xb[(1 - kh) * SK : (2 - kh) * SK]], axis=0
        )
        # [p, g, c, s] = xT[c*128+p, g*512+s]: slice-contiguous per partition
        xt = np.ascontiguousarray(
            xk.T.astype(np.float16)
            .reshape(NDC, 128, NRB, RB)
            .transpose(1, 2, 0, 3)
        )
        in_maps.append(
            {
                "xt": xt,
                "wq": Wq,
                "wk": Wk,
                "wv": Wv,
                "ident": ident,
                "ones": ones,
            }
        )
    return in_maps


def kernel(inputs, Wq, Wk, Wv):
    nc = _get_nc()
    in_maps = make_in_maps(inputs, Wq, Wk, Wv)

    res = run_bass_kernel_spmd(nc, in_maps, core_ids=list(range(NCORES)))

    out = np.empty((B, S, H), dtype=np.float32)
    for b in range(B):
        num = np.zeros((H, S), dtype=np.float32)
        den = np.zeros((1, S), dtype=np.float32)
        for kh in range(2):
            c = 2 * b + kh
            outT = res.results[c]["outT"]  # [H, S], query order permuted
            l = res.results[c]["l"]  # [1, S]
            # queries were ordered [kh-half, other-half]; map back
            perm = np.concatenate(
                [
                    np.arange(kh * SK, (kh + 1) * SK),
                    np.arange((1 - kh) * SK, (2 - kh) * SK),
                ]
            )
            num[:, perm] += outT
            den[:, perm] += l
        out[b] = (num / den).T
    return out
